# revision 3
# baseline (speedup 1.0000x reference)
"""Trainium2 Bass kernel for nn_ContrastiveLoss (InfoNCE-style loss).

Sharding: data-parallel over nodes N=200000 across 8 NeuronCores
(25000 nodes/core, padded to 25088 = 196 tiles x 128 partitions).
embeddings + negative_embs tables are replicated per core; per-core
int32 index tensors are prepared on host. Each core returns per-node
scores [128, 196]; the host masks padding and takes the global mean.

Per node-tile (128 nodes) on device:
  - contiguous load of the 128 e-rows (HWDGE)
  - 11 indirect-DMA gathers ([128,1] row offsets): pos row from
    embeddings, 10 neg rows from negative_embs -> samples [128,11,128]
  - DVE: prod = e (broadcast) * samples; segmented reduce -> dots[128,11]
  - logsumexp: rowmax, subtract, ACT Exp(scale=1/tau) with accumulated
    sum, ACT Ln, score = (m - dots0)/tau + ln(sum)
A drain barrier every 2 tiles works around SWDGE ring-bookkeeping
corruption seen with long unthrottled indirect-DMA pipelines.
"""

import numpy as np

TAU = 0.65
NUM_NEG = 10
N, M, D = 200000, 200000, 128
N_CORES = 8
N_SHARD = N // N_CORES          # 25000
TILES = 196                     # ceil(25000/128)
N_PAD = TILES * 128             # 25088
BARRIER_EVERY = 2               # tiles between drain barriers

_COMPILED = None


def _build_nc():
    import concourse.bass as bass
    import concourse.bacc as bacc
    import concourse.tile as tile
    from concourse import mybir

    F32 = mybir.dt.float32
    I32 = mybir.dt.int32
    AF = mybir.ActivationFunctionType
    OP = mybir.AluOpType

    nc = bacc.Bacc("TRN2", target_bir_lowering=False, debug=False,
                   num_devices=N_CORES)
    emb = nc.dram_tensor("emb", [N, D], F32, kind="ExternalInput").ap()
    negt = nc.dram_tensor("negt", [M, D], F32, kind="ExternalInput").ap()
    esh = nc.dram_tensor("esh", [N_PAD, D], F32, kind="ExternalInput").ap()
    idx = nc.dram_tensor("idx", [128, NUM_NEG + 1, TILES], I32,
                         kind="ExternalInput").ap()
    scores_out = nc.dram_tensor("scores", [128, TILES], F32,
                                kind="ExternalOutput").ap()

    with tile.TileContext(nc) as tc:
        with tc.tile_pool(name="consts", bufs=1) as consts, \
             tc.tile_pool(name="eb", bufs=3) as eb, \
             tc.tile_pool(name="xs", bufs=3) as xs, \
             tc.tile_pool(name="pr", bufs=2) as pr, \
             tc.tile_pool(name="sm", bufs=3) as sm, \
             tc.tile_pool(name="outp", bufs=1) as outp:
            idx_sb = consts.tile([128, NUM_NEG + 1, TILES], I32)
            nc.sync.dma_start(out=idx_sb[:], in_=idx[:])
            scores_sb = outp.tile([128, TILES], F32)

            for t in range(TILES):
                e_t = eb.tile([128, D], F32, tag="e")
                nc.sync.dma_start(out=e_t[:], in_=esh[t * 128:(t + 1) * 128, :])
                samp = xs.tile([128, NUM_NEG + 1, D], F32, tag="s")
                for s in range(NUM_NEG + 1):
                    nc.gpsimd.indirect_dma_start(
                        out=samp[:, s, :],
                        out_offset=None,
                        in_=(emb if s == 0 else negt)[:, :],
                        in_offset=bass.IndirectOffsetOnAxis(
                            ap=idx_sb[:, s, t:t + 1], axis=0),
                    )
                prod = pr.tile([128, NUM_NEG + 1, D], F32, tag="p")
                nc.vector.tensor_tensor(
                    out=prod[:], in0=samp[:],
                    in1=e_t[:].rearrange("p (o d) -> p o d", o=1).to_broadcast(
                        [128, NUM_NEG + 1, D]),
                    op=OP.mult)
                dots = sm.tile([128, NUM_NEG + 1], F32, tag="d")
                nc.vector.tensor_reduce(
                    out=dots[:], in_=prod[:], axis=mybir.AxisListType.X,
                    op=OP.add)
                m = sm.tile([128, 1], F32, tag="m")
                nc.vector.tensor_reduce(
                    out=m[:], in_=dots[:], axis=mybir.AxisListType.X,
                    op=OP.max)
                subx = sm.tile([128, NUM_NEG + 1], F32, tag="sub")
                nc.vector.tensor_tensor(
                    out=subx[:], in0=dots[:],
                    in1=m[:].to_broadcast([128, NUM_NEG + 1]), op=OP.subtract)
                expd = sm.tile([128, NUM_NEG + 1], F32, tag="ex")
                ssum = sm.tile([128, 1], F32, tag="ss")
                nc.scalar.activation(out=expd[:], in_=subx[:], func=AF.Exp,
                                     scale=1.0 / TAU, accum_out=ssum[:])
                lnv = sm.tile([128, 1], F32, tag="ln")
                nc.scalar.activation(out=lnv[:], in_=ssum[:], func=AF.Ln)
                t1 = sm.tile([128, 1], F32, tag="t1")
                nc.vector.tensor_tensor(out=t1[:], in0=m[:], in1=dots[:, 0:1],
                                        op=OP.subtract)
                nc.vector.scalar_tensor_tensor(
                    out=scores_sb[:, t:t + 1], in0=t1[:], scalar=1.0 / TAU,
                    in1=lnv[:], op0=OP.mult, op1=OP.add)
                if (t + 1) % BARRIER_EVERY == 0:
                    tc.strict_bb_all_engine_barrier()

            nc.sync.dma_start(out=scores_out[:], in_=scores_sb[:])
    nc.compile()
    return nc


class _Runner:
    """Compile-once SPMD runner over axon PJRT (8 cores, shard_map)."""

    def __init__(self, nc):
        import jax
        import numpy as np
        from jax.sharding import Mesh, PartitionSpec
        from jax.experimental.shard_map import shard_map
        from concourse import bass2jax, mybir
        from concourse.bass2jax import _bass_exec_p, install_neuronx_cc_hook

        install_neuronx_cc_hook()
        self.jax = jax
        partition_name = (
            nc.partition_id_tensor.name if nc.partition_id_tensor else None
        )
        in_names, out_names, out_avals, zero_outs = [], [], [], []
        for alloc in nc.m.functions[0].allocations:
            if not isinstance(alloc, mybir.MemoryLocationSet):
                continue
            name = alloc.memorylocations[0].name
            if alloc.kind == "ExternalInput":
                if name != partition_name:
                    in_names.append(name)
            elif alloc.kind == "ExternalOutput":
                shape = tuple(alloc.tensor_shape)
                dtype = mybir.dt.np(alloc.dtype)
                out_names.append(name)
                out_avals.append(jax.core.ShapedArray(shape, dtype))
                zero_outs.append(np.zeros(shape, dtype))
        self.in_names, self.out_names = in_names, out_names
        self.out_avals, self.zero_outs = out_avals, zero_outs
        n_params, n_outs = len(in_names), len(out_names)
        all_in_names = in_names + out_names
        if partition_name is not None:
            all_in_names.append(partition_name)
        donate = tuple(range(n_params, n_params + n_outs))

        def _body(*args):
            operands = list(args)
            if partition_name is not None:
                operands.append(bass2jax.partition_id_tensor())
            outs = _bass_exec_p.bind(
                *operands,
                out_avals=tuple(out_avals),
                in_names=tuple(all_in_names),
                out_names=tuple(out_names),
                lowering_input_output_aliases=(),
                sim_require_finite=True,
                sim_require_nnan=True,
                nc=nc,
            )
            return tuple(outs)

        devices = jax.devices()[:N_CORES]
        self.mesh = Mesh(np.asarray(devices), ("core",))
        in_specs = (PartitionSpec("core"),) * (n_params + n_outs)
        out_specs = (PartitionSpec("core"),) * n_outs
        self.fn = jax.jit(
            shard_map(_body, mesh=self.mesh, in_specs=in_specs,
                      out_specs=out_specs, check_rep=False),
            donate_argnums=donate, keep_unused=True)
        self._staged = None

    def stage(self, in_maps):
        from jax.sharding import NamedSharding, PartitionSpec
        concat = [
            np.ascontiguousarray(
                np.concatenate([np.asarray(m[n]) for m in in_maps], axis=0))
            for n in self.in_names
        ]
        sh = NamedSharding(self.mesh, PartitionSpec("core"))
        self._staged = [self.jax.device_put(a, sh) for a in concat]
        self.jax.block_until_ready(self._staged)

    def run(self):
        zeros = [
            np.zeros((N_CORES * z.shape[0], *z.shape[1:]), z.dtype)
            for z in self.zero_outs
        ]
        out = self.fn(*self._staged, *zeros)
        self.jax.block_until_ready(out)
        return [
            {
                n: np.asarray(out[i]).reshape(
                    N_CORES, *self.out_avals[i].shape)[c]
                for i, n in enumerate(self.out_names)
            }
            for c in range(N_CORES)
        ]


def _prep_inputs(embeddings, negative_embs, community_pos_options, neg_idx,
                 iter_n):
    embeddings = np.ascontiguousarray(np.asarray(embeddings, dtype=np.float32))
    negative_embs = np.ascontiguousarray(
        np.asarray(negative_embs, dtype=np.float32))
    cpo = np.asarray(community_pos_options)
    nidx = np.asarray(neg_idx)
    it = int(np.asarray(iter_n))
    pos_idx = cpo[:, it - 1].astype(np.int32)          # [N] rows of embeddings
    nidx = nidx.astype(np.int32)                       # [NUM_NEG, N]

    in_maps = []
    for c in range(N_CORES):
        base = c * N_SHARD
        esh = np.zeros((N_PAD, D), np.float32)
        esh[:N_SHARD] = embeddings[base:base + N_SHARD]
        pos_pad = np.zeros(N_PAD, np.int32)
        pos_pad[:N_SHARD] = pos_idx[base:base + N_SHARD]
        neg_pad = np.zeros((NUM_NEG, N_PAD), np.int32)
        neg_pad[:, :N_SHARD] = nidx[:, base:base + N_SHARD]
        idx = np.empty((128, NUM_NEG + 1, TILES), np.int32)
        # node j = t*128 + p  ->  idx[p, s, t]
        idx[:, 0, :] = pos_pad.reshape(TILES, 128).T
        idx[:, 1:, :] = neg_pad.reshape(NUM_NEG, TILES, 128).transpose(2, 0, 1)
        in_maps.append({
            "emb": embeddings,
            "negt": negative_embs,
            "esh": esh,
            "idx": idx,
        })
    return in_maps


def _get_runner():
    global _COMPILED
    if _COMPILED is None:
        _COMPILED = _Runner(_build_nc())
    return _COMPILED


def kernel(embeddings, negative_embs, community_pos_options, neg_idx, iter_n):
    r = _get_runner()
    in_maps = _prep_inputs(embeddings, negative_embs, community_pos_options,
                           neg_idx, iter_n)
    r.stage(in_maps)
    res = r.run()
    total = 0.0
    for c in range(N_CORES):
        sc = res[c]["scores"]              # [128, TILES], node j at [j%128, j//128]
        flat = sc.T.reshape(-1)            # node-ordered
        total += float(flat[:N_SHARD].astype(np.float64).sum())
    return np.float32(total / N)


# revision 4
# speedup vs baseline: 1.4901x; 1.4901x over previous
"""Trainium2 Bass kernel for nn_ContrastiveLoss (InfoNCE-style loss).

Sharding: data-parallel over nodes N=200000 across 8 NeuronCores
(25000 nodes/core, padded to 25088 = 196 tiles x 128 partitions).
embeddings + negative_embs tables are replicated per core; per-core
int32 index tensors are prepared on host. Each core returns per-node
scores [128, 196]; the host masks padding and takes the global mean.

Per node-tile (128 nodes) on device:
  - contiguous load of the 128 e-rows (HWDGE)
  - 11 indirect-DMA gathers ([128,1] row offsets): pos row from
    embeddings, 10 neg rows from negative_embs -> samples [128,11,128]
  - DVE: prod = e (broadcast) * samples; segmented reduce -> dots[128,11]
  - logsumexp: rowmax, subtract, ACT Exp(scale=1/tau) with accumulated
    sum, ACT Ln, score = (m - dots0)/tau + ln(sum)
A drain barrier every 2 tiles works around SWDGE ring-bookkeeping
corruption seen with long unthrottled indirect-DMA pipelines.
"""

import numpy as np

TAU = 0.65
NUM_NEG = 10
N, M, D = 200000, 200000, 128
N_CORES = 8
N_SHARD = N // N_CORES          # 25000
TILES = 196                     # ceil(25000/128)
N_PAD = TILES * 128             # 25088
BARRIER_EVERY = 4               # tiles between drain barriers

_COMPILED = None


def _build_nc():
    import concourse.bass as bass
    import concourse.bacc as bacc
    import concourse.tile as tile
    from concourse import mybir

    F32 = mybir.dt.float32
    I32 = mybir.dt.int32
    AF = mybir.ActivationFunctionType
    OP = mybir.AluOpType

    nc = bacc.Bacc("TRN2", target_bir_lowering=False, debug=False,
                   num_devices=N_CORES)
    emb = nc.dram_tensor("emb", [N, D], F32, kind="ExternalInput").ap()
    negt = nc.dram_tensor("negt", [M, D], F32, kind="ExternalInput").ap()
    esh = nc.dram_tensor("esh", [N_PAD, D], F32, kind="ExternalInput").ap()
    idx = nc.dram_tensor("idx", [128, NUM_NEG + 1, TILES], I32,
                         kind="ExternalInput").ap()
    scores_out = nc.dram_tensor("scores", [128, TILES], F32,
                                kind="ExternalOutput").ap()

    with tile.TileContext(nc) as tc:
        with tc.tile_pool(name="consts", bufs=1) as consts, \
             tc.tile_pool(name="eb", bufs=5) as eb, \
             tc.tile_pool(name="xs", bufs=5) as xs, \
             tc.tile_pool(name="pr", bufs=2) as pr, \
             tc.tile_pool(name="sm", bufs=3) as sm, \
             tc.tile_pool(name="outp", bufs=1) as outp:
            idx_sb = consts.tile([128, NUM_NEG + 1, TILES], I32)
            nc.sync.dma_start(out=idx_sb[:], in_=idx[:])
            scores_sb = outp.tile([128, TILES], F32)

            for t in range(TILES):
                e_t = eb.tile([128, D], F32, tag="e")
                nc.sync.dma_start(out=e_t[:], in_=esh[t * 128:(t + 1) * 128, :])
                samp = xs.tile([128, NUM_NEG + 1, D], F32, tag="s")
                for s in range(NUM_NEG + 1):
                    nc.gpsimd.indirect_dma_start(
                        out=samp[:, s, :],
                        out_offset=None,
                        in_=(emb if s == 0 else negt)[:, :],
                        in_offset=bass.IndirectOffsetOnAxis(
                            ap=idx_sb[:, s, t:t + 1], axis=0),
                    )
                prod = pr.tile([128, NUM_NEG + 1, D], F32, tag="p")
                nc.vector.tensor_tensor(
                    out=prod[:], in0=samp[:],
                    in1=e_t[:].rearrange("p (o d) -> p o d", o=1).to_broadcast(
                        [128, NUM_NEG + 1, D]),
                    op=OP.mult)
                dots = sm.tile([128, NUM_NEG + 1], F32, tag="d")
                nc.vector.tensor_reduce(
                    out=dots[:], in_=prod[:], axis=mybir.AxisListType.X,
                    op=OP.add)
                m = sm.tile([128, 1], F32, tag="m")
                nc.vector.tensor_reduce(
                    out=m[:], in_=dots[:], axis=mybir.AxisListType.X,
                    op=OP.max)
                subx = sm.tile([128, NUM_NEG + 1], F32, tag="sub")
                nc.vector.tensor_tensor(
                    out=subx[:], in0=dots[:],
                    in1=m[:].to_broadcast([128, NUM_NEG + 1]), op=OP.subtract)
                expd = sm.tile([128, NUM_NEG + 1], F32, tag="ex")
                ssum = sm.tile([128, 1], F32, tag="ss")
                nc.scalar.activation(out=expd[:], in_=subx[:], func=AF.Exp,
                                     scale=1.0 / TAU, accum_out=ssum[:])
                lnv = sm.tile([128, 1], F32, tag="ln")
                nc.scalar.activation(out=lnv[:], in_=ssum[:], func=AF.Ln)
                t1 = sm.tile([128, 1], F32, tag="t1")
                nc.vector.tensor_tensor(out=t1[:], in0=m[:], in1=dots[:, 0:1],
                                        op=OP.subtract)
                nc.vector.scalar_tensor_tensor(
                    out=scores_sb[:, t:t + 1], in0=t1[:], scalar=1.0 / TAU,
                    in1=lnv[:], op0=OP.mult, op1=OP.add)
                if (t + 1) % BARRIER_EVERY == 0:
                    tc.strict_bb_all_engine_barrier()

            nc.sync.dma_start(out=scores_out[:], in_=scores_sb[:])
    nc.compile()
    return nc


class _Runner:
    """Compile-once SPMD runner over axon PJRT (8 cores, shard_map)."""

    def __init__(self, nc):
        import jax
        import numpy as np
        from jax.sharding import Mesh, PartitionSpec
        from jax.experimental.shard_map import shard_map
        from concourse import bass2jax, mybir
        from concourse.bass2jax import _bass_exec_p, install_neuronx_cc_hook

        install_neuronx_cc_hook()
        self.jax = jax
        partition_name = (
            nc.partition_id_tensor.name if nc.partition_id_tensor else None
        )
        in_names, out_names, out_avals, zero_outs = [], [], [], []
        for alloc in nc.m.functions[0].allocations:
            if not isinstance(alloc, mybir.MemoryLocationSet):
                continue
            name = alloc.memorylocations[0].name
            if alloc.kind == "ExternalInput":
                if name != partition_name:
                    in_names.append(name)
            elif alloc.kind == "ExternalOutput":
                shape = tuple(alloc.tensor_shape)
                dtype = mybir.dt.np(alloc.dtype)
                out_names.append(name)
                out_avals.append(jax.core.ShapedArray(shape, dtype))
                zero_outs.append(np.zeros(shape, dtype))
        self.in_names, self.out_names = in_names, out_names
        self.out_avals, self.zero_outs = out_avals, zero_outs
        n_params, n_outs = len(in_names), len(out_names)
        all_in_names = in_names + out_names
        if partition_name is not None:
            all_in_names.append(partition_name)
        donate = tuple(range(n_params, n_params + n_outs))

        def _body(*args):
            operands = list(args)
            if partition_name is not None:
                operands.append(bass2jax.partition_id_tensor())
            outs = _bass_exec_p.bind(
                *operands,
                out_avals=tuple(out_avals),
                in_names=tuple(all_in_names),
                out_names=tuple(out_names),
                lowering_input_output_aliases=(),
                sim_require_finite=True,
                sim_require_nnan=True,
                nc=nc,
            )
            return tuple(outs)

        devices = jax.devices()[:N_CORES]
        self.mesh = Mesh(np.asarray(devices), ("core",))
        in_specs = (PartitionSpec("core"),) * (n_params + n_outs)
        out_specs = (PartitionSpec("core"),) * n_outs
        self.fn = jax.jit(
            shard_map(_body, mesh=self.mesh, in_specs=in_specs,
                      out_specs=out_specs, check_rep=False),
            donate_argnums=donate, keep_unused=True)
        self._staged = None

    def stage(self, in_maps):
        from jax.sharding import NamedSharding, PartitionSpec
        concat = [
            np.ascontiguousarray(
                np.concatenate([np.asarray(m[n]) for m in in_maps], axis=0))
            for n in self.in_names
        ]
        sh = NamedSharding(self.mesh, PartitionSpec("core"))
        self._staged = [self.jax.device_put(a, sh) for a in concat]
        self.jax.block_until_ready(self._staged)

    def run(self):
        zeros = [
            np.zeros((N_CORES * z.shape[0], *z.shape[1:]), z.dtype)
            for z in self.zero_outs
        ]
        out = self.fn(*self._staged, *zeros)
        self.jax.block_until_ready(out)
        return [
            {
                n: np.asarray(out[i]).reshape(
                    N_CORES, *self.out_avals[i].shape)[c]
                for i, n in enumerate(self.out_names)
            }
            for c in range(N_CORES)
        ]


def _prep_inputs(embeddings, negative_embs, community_pos_options, neg_idx,
                 iter_n):
    embeddings = np.ascontiguousarray(np.asarray(embeddings, dtype=np.float32))
    negative_embs = np.ascontiguousarray(
        np.asarray(negative_embs, dtype=np.float32))
    cpo = np.asarray(community_pos_options)
    nidx = np.asarray(neg_idx)
    it = int(np.asarray(iter_n))
    pos_idx = cpo[:, it - 1].astype(np.int32)          # [N] rows of embeddings
    nidx = nidx.astype(np.int32)                       # [NUM_NEG, N]

    in_maps = []
    for c in range(N_CORES):
        base = c * N_SHARD
        esh = np.zeros((N_PAD, D), np.float32)
        esh[:N_SHARD] = embeddings[base:base + N_SHARD]
        pos_pad = np.zeros(N_PAD, np.int32)
        pos_pad[:N_SHARD] = pos_idx[base:base + N_SHARD]
        neg_pad = np.zeros((NUM_NEG, N_PAD), np.int32)
        neg_pad[:, :N_SHARD] = nidx[:, base:base + N_SHARD]
        idx = np.empty((128, NUM_NEG + 1, TILES), np.int32)
        # node j = t*128 + p  ->  idx[p, s, t]
        idx[:, 0, :] = pos_pad.reshape(TILES, 128).T
        idx[:, 1:, :] = neg_pad.reshape(NUM_NEG, TILES, 128).transpose(2, 0, 1)
        in_maps.append({
            "emb": embeddings,
            "negt": negative_embs,
            "esh": esh,
            "idx": idx,
        })
    return in_maps


def _get_runner():
    global _COMPILED
    if _COMPILED is None:
        _COMPILED = _Runner(_build_nc())
    return _COMPILED


def kernel(embeddings, negative_embs, community_pos_options, neg_idx, iter_n):
    r = _get_runner()
    in_maps = _prep_inputs(embeddings, negative_embs, community_pos_options,
                           neg_idx, iter_n)
    r.stage(in_maps)
    res = r.run()
    total = 0.0
    for c in range(N_CORES):
        sc = res[c]["scores"]              # [128, TILES], node j at [j%128, j//128]
        flat = sc.T.reshape(-1)            # node-ordered
        total += float(flat[:N_SHARD].astype(np.float64).sum())
    return np.float32(total / N)


# revision 5
# speedup vs baseline: 2.4058x; 1.6146x over previous
"""Trainium2 Bass kernel for nn_ContrastiveLoss (InfoNCE-style loss).

Sharding: data-parallel over nodes N=200000 across 8 NeuronCores
(25000 nodes/core, padded to 25088 = 196 tiles x 128 partitions).
embeddings + negative_embs tables are replicated per core; per-core
int32 index tensors are prepared on host. Each core returns per-node
scores [128, 196]; the host masks padding and takes the global mean.

Per node-tile (128 nodes) on device:
  - contiguous load of the 128 e-rows (HWDGE)
  - 11 indirect-DMA gathers ([128,1] row offsets): pos row from
    embeddings, 10 neg rows from negative_embs -> samples [128,11,128]
  - DVE: prod = e (broadcast) * samples; segmented reduce -> dots[128,11]
  - logsumexp: rowmax, subtract, ACT Exp(scale=1/tau) with accumulated
    sum, ACT Ln, score = (m - dots0)/tau + ln(sum)
A drain barrier every 2 tiles works around SWDGE ring-bookkeeping
corruption seen with long unthrottled indirect-DMA pipelines.
"""

import numpy as np

TAU = 0.65
NUM_NEG = 10
N, M, D = 200000, 200000, 128
N_CORES = 8
N_SHARD = N // N_CORES          # 25000
TILES = 196                     # ceil(25000/128)
N_PAD = TILES * 128             # 25088
BARRIER_EVERY = 2               # tiles between drain barriers

_COMPILED = None


def _build_nc():
    import concourse.bass as bass
    import concourse.bacc as bacc
    import concourse.tile as tile
    from concourse import mybir

    F32 = mybir.dt.float32
    I32 = mybir.dt.int32
    AF = mybir.ActivationFunctionType
    OP = mybir.AluOpType

    nc = bacc.Bacc("TRN2", target_bir_lowering=False, debug=False,
                   num_devices=N_CORES)
    emb = nc.dram_tensor("emb", [N, D], F32, kind="ExternalInput").ap()
    negt = nc.dram_tensor("negt", [M, D], F32, kind="ExternalInput").ap()
    esh = nc.dram_tensor("esh", [N_PAD, D], F32, kind="ExternalInput").ap()
    idx = nc.dram_tensor("idx", [128, NUM_NEG + 1, TILES], I32,
                         kind="ExternalInput").ap()
    scores_out = nc.dram_tensor("scores", [128, TILES], F32,
                                kind="ExternalOutput").ap()

    with tile.TileContext(nc) as tc:
        with tc.tile_pool(name="consts", bufs=1) as consts, \
             tc.tile_pool(name="eb", bufs=5) as eb, \
             tc.tile_pool(name="xs", bufs=5) as xs, \
             tc.tile_pool(name="pr", bufs=2) as pr, \
             tc.tile_pool(name="sm", bufs=3) as sm, \
             tc.tile_pool(name="outp", bufs=1) as outp:
            idx_sb = consts.tile([128, NUM_NEG + 1, TILES], I32)
            nc.sync.dma_start(out=idx_sb[:], in_=idx[:])
            scores_sb = outp.tile([128, TILES], F32)

            for t in range(TILES):
                e_t = eb.tile([128, D], F32, tag="e")
                nc.sync.dma_start(out=e_t[:], in_=esh[t * 128:(t + 1) * 128, :])
                samp = xs.tile([128, NUM_NEG + 1, D], F32, tag="s")
                for s in range(NUM_NEG + 1):
                    nc.gpsimd.indirect_dma_start(
                        out=samp[:, s, :],
                        out_offset=None,
                        in_=(emb if s == 0 else negt)[:, :],
                        in_offset=bass.IndirectOffsetOnAxis(
                            ap=idx_sb[:, s, t:t + 1], axis=0),
                    )
                prod = pr.tile([128, NUM_NEG + 1, D], F32, tag="p")
                nc.vector.tensor_tensor(
                    out=prod[:], in0=samp[:],
                    in1=e_t[:].rearrange("p (o d) -> p o d", o=1).to_broadcast(
                        [128, NUM_NEG + 1, D]),
                    op=OP.mult)
                dots = sm.tile([128, NUM_NEG + 1], F32, tag="d")
                nc.vector.tensor_reduce(
                    out=dots[:], in_=prod[:], axis=mybir.AxisListType.X,
                    op=OP.add)
                m = sm.tile([128, 1], F32, tag="m")
                nc.vector.tensor_reduce(
                    out=m[:], in_=dots[:], axis=mybir.AxisListType.X,
                    op=OP.max)
                subx = sm.tile([128, NUM_NEG + 1], F32, tag="sub")
                nc.vector.tensor_tensor(
                    out=subx[:], in0=dots[:],
                    in1=m[:].to_broadcast([128, NUM_NEG + 1]), op=OP.subtract)
                expd = sm.tile([128, NUM_NEG + 1], F32, tag="ex")
                ssum = sm.tile([128, 1], F32, tag="ss")
                nc.scalar.activation(out=expd[:], in_=subx[:], func=AF.Exp,
                                     scale=1.0 / TAU, accum_out=ssum[:])
                lnv = sm.tile([128, 1], F32, tag="ln")
                nc.scalar.activation(out=lnv[:], in_=ssum[:], func=AF.Ln)
                t1 = sm.tile([128, 1], F32, tag="t1")
                nc.vector.tensor_tensor(out=t1[:], in0=m[:], in1=dots[:, 0:1],
                                        op=OP.subtract)
                nc.vector.scalar_tensor_tensor(
                    out=scores_sb[:, t:t + 1], in0=t1[:], scalar=1.0 / TAU,
                    in1=lnv[:], op0=OP.mult, op1=OP.add)
                if (t + 1) % BARRIER_EVERY == 0:
                    tc.no_sync_barrier()

            nc.sync.dma_start(out=scores_out[:], in_=scores_sb[:])
    nc.compile()
    return nc


class _Runner:
    """Compile-once SPMD runner over axon PJRT (8 cores, shard_map)."""

    def __init__(self, nc):
        import jax
        import numpy as np
        from jax.sharding import Mesh, PartitionSpec
        from jax.experimental.shard_map import shard_map
        from concourse import bass2jax, mybir
        from concourse.bass2jax import _bass_exec_p, install_neuronx_cc_hook

        install_neuronx_cc_hook()
        self.jax = jax
        partition_name = (
            nc.partition_id_tensor.name if nc.partition_id_tensor else None
        )
        in_names, out_names, out_avals, zero_outs = [], [], [], []
        for alloc in nc.m.functions[0].allocations:
            if not isinstance(alloc, mybir.MemoryLocationSet):
                continue
            name = alloc.memorylocations[0].name
            if alloc.kind == "ExternalInput":
                if name != partition_name:
                    in_names.append(name)
            elif alloc.kind == "ExternalOutput":
                shape = tuple(alloc.tensor_shape)
                dtype = mybir.dt.np(alloc.dtype)
                out_names.append(name)
                out_avals.append(jax.core.ShapedArray(shape, dtype))
                zero_outs.append(np.zeros(shape, dtype))
        self.in_names, self.out_names = in_names, out_names
        self.out_avals, self.zero_outs = out_avals, zero_outs
        n_params, n_outs = len(in_names), len(out_names)
        all_in_names = in_names + out_names
        if partition_name is not None:
            all_in_names.append(partition_name)
        donate = tuple(range(n_params, n_params + n_outs))

        def _body(*args):
            operands = list(args)
            if partition_name is not None:
                operands.append(bass2jax.partition_id_tensor())
            outs = _bass_exec_p.bind(
                *operands,
                out_avals=tuple(out_avals),
                in_names=tuple(all_in_names),
                out_names=tuple(out_names),
                lowering_input_output_aliases=(),
                sim_require_finite=True,
                sim_require_nnan=True,
                nc=nc,
            )
            return tuple(outs)

        devices = jax.devices()[:N_CORES]
        self.mesh = Mesh(np.asarray(devices), ("core",))
        in_specs = (PartitionSpec("core"),) * (n_params + n_outs)
        out_specs = (PartitionSpec("core"),) * n_outs
        self.fn = jax.jit(
            shard_map(_body, mesh=self.mesh, in_specs=in_specs,
                      out_specs=out_specs, check_rep=False),
            donate_argnums=donate, keep_unused=True)
        self._staged = None

    def stage(self, in_maps):
        from jax.sharding import NamedSharding, PartitionSpec
        concat = [
            np.ascontiguousarray(
                np.concatenate([np.asarray(m[n]) for m in in_maps], axis=0))
            for n in self.in_names
        ]
        sh = NamedSharding(self.mesh, PartitionSpec("core"))
        self._staged = [self.jax.device_put(a, sh) for a in concat]
        self.jax.block_until_ready(self._staged)

    def run(self):
        zeros = [
            np.zeros((N_CORES * z.shape[0], *z.shape[1:]), z.dtype)
            for z in self.zero_outs
        ]
        out = self.fn(*self._staged, *zeros)
        self.jax.block_until_ready(out)
        return [
            {
                n: np.asarray(out[i]).reshape(
                    N_CORES, *self.out_avals[i].shape)[c]
                for i, n in enumerate(self.out_names)
            }
            for c in range(N_CORES)
        ]


def _prep_inputs(embeddings, negative_embs, community_pos_options, neg_idx,
                 iter_n):
    embeddings = np.ascontiguousarray(np.asarray(embeddings, dtype=np.float32))
    negative_embs = np.ascontiguousarray(
        np.asarray(negative_embs, dtype=np.float32))
    cpo = np.asarray(community_pos_options)
    nidx = np.asarray(neg_idx)
    it = int(np.asarray(iter_n))
    pos_idx = cpo[:, it - 1].astype(np.int32)          # [N] rows of embeddings
    nidx = nidx.astype(np.int32)                       # [NUM_NEG, N]

    in_maps = []
    for c in range(N_CORES):
        base = c * N_SHARD
        esh = np.zeros((N_PAD, D), np.float32)
        esh[:N_SHARD] = embeddings[base:base + N_SHARD]
        pos_pad = np.zeros(N_PAD, np.int32)
        pos_pad[:N_SHARD] = pos_idx[base:base + N_SHARD]
        neg_pad = np.zeros((NUM_NEG, N_PAD), np.int32)
        neg_pad[:, :N_SHARD] = nidx[:, base:base + N_SHARD]
        idx = np.empty((128, NUM_NEG + 1, TILES), np.int32)
        # node j = t*128 + p  ->  idx[p, s, t]
        idx[:, 0, :] = pos_pad.reshape(TILES, 128).T
        idx[:, 1:, :] = neg_pad.reshape(NUM_NEG, TILES, 128).transpose(2, 0, 1)
        in_maps.append({
            "emb": embeddings,
            "negt": negative_embs,
            "esh": esh,
            "idx": idx,
        })
    return in_maps


def _get_runner():
    global _COMPILED
    if _COMPILED is None:
        _COMPILED = _Runner(_build_nc())
    return _COMPILED


def kernel(embeddings, negative_embs, community_pos_options, neg_idx, iter_n):
    r = _get_runner()
    in_maps = _prep_inputs(embeddings, negative_embs, community_pos_options,
                           neg_idx, iter_n)
    r.stage(in_maps)
    res = r.run()
    total = 0.0
    for c in range(N_CORES):
        sc = res[c]["scores"]              # [128, TILES], node j at [j%128, j//128]
        flat = sc.T.reshape(-1)            # node-ordered
        total += float(flat[:N_SHARD].astype(np.float64).sum())
    return np.float32(total / N)


# revision 6
# speedup vs baseline: 2.6939x; 1.1198x over previous
"""Trainium2 Bass kernel for nn_ContrastiveLoss (InfoNCE-style loss).

Sharding: data-parallel over nodes N=200000 across 8 NeuronCores
(25000 nodes/core, padded to 25088 = 196 tiles x 128 partitions).
embeddings + negative_embs tables are replicated per core; per-core
int32 index tensors are prepared on host. Each core returns per-node
scores [128, 196]; the host masks padding and takes the global mean.

Per node-tile (128 nodes) on device:
  - contiguous load of the 128 e-rows (HWDGE)
  - 11 indirect-DMA gathers ([128,1] row offsets): pos row from
    embeddings, 10 neg rows from negative_embs -> samples [128,11,128]
  - DVE: prod = e (broadcast) * samples; segmented reduce -> dots[128,11]
  - logsumexp: rowmax, subtract, ACT Exp(scale=1/tau) with accumulated
    sum, ACT Ln, score = (m - dots0)/tau + ln(sum)
A drain barrier every 2 tiles works around SWDGE ring-bookkeeping
corruption seen with long unthrottled indirect-DMA pipelines.
"""

import numpy as np

TAU = 0.65
NUM_NEG = 10
N, M, D = 200000, 200000, 128
N_CORES = 8
N_SHARD = N // N_CORES          # 25000
TILES = 196                     # ceil(25000/128)
N_PAD = TILES * 128             # 25088
BARRIER_EVERY = 3               # tiles between drain barriers

_COMPILED = None


def _build_nc():
    import concourse.bass as bass
    import concourse.bacc as bacc
    import concourse.tile as tile
    from concourse import mybir

    F32 = mybir.dt.float32
    I32 = mybir.dt.int32
    AF = mybir.ActivationFunctionType
    OP = mybir.AluOpType

    nc = bacc.Bacc("TRN2", target_bir_lowering=False, debug=False,
                   num_devices=N_CORES)
    emb = nc.dram_tensor("emb", [N, D], F32, kind="ExternalInput").ap()
    negt = nc.dram_tensor("negt", [M, D], F32, kind="ExternalInput").ap()
    esh = nc.dram_tensor("esh", [N_PAD, D], F32, kind="ExternalInput").ap()
    idx = nc.dram_tensor("idx", [128, NUM_NEG + 1, TILES], I32,
                         kind="ExternalInput").ap()
    scores_out = nc.dram_tensor("scores", [128, TILES], F32,
                                kind="ExternalOutput").ap()

    with tile.TileContext(nc) as tc:
        with tc.tile_pool(name="consts", bufs=1) as consts, \
             tc.tile_pool(name="eb", bufs=5) as eb, \
             tc.tile_pool(name="xs", bufs=8) as xs, \
             tc.tile_pool(name="pr", bufs=3) as pr, \
             tc.tile_pool(name="sm", bufs=4) as sm, \
             tc.tile_pool(name="outp", bufs=1) as outp:
            idx_sb = consts.tile([128, NUM_NEG + 1, TILES], I32)
            nc.sync.dma_start(out=idx_sb[:], in_=idx[:])
            scores_sb = outp.tile([128, TILES], F32)

            for t in range(TILES):
                e_t = eb.tile([128, D], F32, tag="e")
                nc.sync.dma_start(out=e_t[:], in_=esh[t * 128:(t + 1) * 128, :])
                samp = xs.tile([128, NUM_NEG + 1, D], F32, tag="s")
                for s in range(NUM_NEG + 1):
                    nc.gpsimd.indirect_dma_start(
                        out=samp[:, s, :],
                        out_offset=None,
                        in_=(emb if s == 0 else negt)[:, :],
                        in_offset=bass.IndirectOffsetOnAxis(
                            ap=idx_sb[:, s, t:t + 1], axis=0),
                    )
                prod = pr.tile([128, NUM_NEG + 1, D], F32, tag="p")
                nc.vector.tensor_tensor(
                    out=prod[:], in0=samp[:],
                    in1=e_t[:].rearrange("p (o d) -> p o d", o=1).to_broadcast(
                        [128, NUM_NEG + 1, D]),
                    op=OP.mult)
                dots = sm.tile([128, NUM_NEG + 1], F32, tag="d")
                nc.vector.tensor_reduce(
                    out=dots[:], in_=prod[:], axis=mybir.AxisListType.X,
                    op=OP.add)
                m = sm.tile([128, 1], F32, tag="m")
                nc.vector.tensor_reduce(
                    out=m[:], in_=dots[:], axis=mybir.AxisListType.X,
                    op=OP.max)
                subx = sm.tile([128, NUM_NEG + 1], F32, tag="sub")
                nc.vector.tensor_tensor(
                    out=subx[:], in0=dots[:],
                    in1=m[:].to_broadcast([128, NUM_NEG + 1]), op=OP.subtract)
                expd = sm.tile([128, NUM_NEG + 1], F32, tag="ex")
                ssum = sm.tile([128, 1], F32, tag="ss")
                nc.scalar.activation(out=expd[:], in_=subx[:], func=AF.Exp,
                                     scale=1.0 / TAU, accum_out=ssum[:])
                lnv = sm.tile([128, 1], F32, tag="ln")
                nc.scalar.activation(out=lnv[:], in_=ssum[:], func=AF.Ln)
                t1 = sm.tile([128, 1], F32, tag="t1")
                nc.vector.tensor_tensor(out=t1[:], in0=m[:], in1=dots[:, 0:1],
                                        op=OP.subtract)
                nc.vector.scalar_tensor_tensor(
                    out=scores_sb[:, t:t + 1], in0=t1[:], scalar=1.0 / TAU,
                    in1=lnv[:], op0=OP.mult, op1=OP.add)
                if (t + 1) % BARRIER_EVERY == 0:
                    tc.no_sync_barrier()

            nc.sync.dma_start(out=scores_out[:], in_=scores_sb[:])
    nc.compile()
    return nc


class _Runner:
    """Compile-once SPMD runner over axon PJRT (8 cores, shard_map)."""

    def __init__(self, nc):
        import jax
        import numpy as np
        from jax.sharding import Mesh, PartitionSpec
        from jax.experimental.shard_map import shard_map
        from concourse import bass2jax, mybir
        from concourse.bass2jax import _bass_exec_p, install_neuronx_cc_hook

        install_neuronx_cc_hook()
        self.jax = jax
        partition_name = (
            nc.partition_id_tensor.name if nc.partition_id_tensor else None
        )
        in_names, out_names, out_avals, zero_outs = [], [], [], []
        for alloc in nc.m.functions[0].allocations:
            if not isinstance(alloc, mybir.MemoryLocationSet):
                continue
            name = alloc.memorylocations[0].name
            if alloc.kind == "ExternalInput":
                if name != partition_name:
                    in_names.append(name)
            elif alloc.kind == "ExternalOutput":
                shape = tuple(alloc.tensor_shape)
                dtype = mybir.dt.np(alloc.dtype)
                out_names.append(name)
                out_avals.append(jax.core.ShapedArray(shape, dtype))
                zero_outs.append(np.zeros(shape, dtype))
        self.in_names, self.out_names = in_names, out_names
        self.out_avals, self.zero_outs = out_avals, zero_outs
        n_params, n_outs = len(in_names), len(out_names)
        all_in_names = in_names + out_names
        if partition_name is not None:
            all_in_names.append(partition_name)
        donate = tuple(range(n_params, n_params + n_outs))

        def _body(*args):
            operands = list(args)
            if partition_name is not None:
                operands.append(bass2jax.partition_id_tensor())
            outs = _bass_exec_p.bind(
                *operands,
                out_avals=tuple(out_avals),
                in_names=tuple(all_in_names),
                out_names=tuple(out_names),
                lowering_input_output_aliases=(),
                sim_require_finite=True,
                sim_require_nnan=True,
                nc=nc,
            )
            return tuple(outs)

        devices = jax.devices()[:N_CORES]
        self.mesh = Mesh(np.asarray(devices), ("core",))
        in_specs = (PartitionSpec("core"),) * (n_params + n_outs)
        out_specs = (PartitionSpec("core"),) * n_outs
        self.fn = jax.jit(
            shard_map(_body, mesh=self.mesh, in_specs=in_specs,
                      out_specs=out_specs, check_rep=False),
            donate_argnums=donate, keep_unused=True)
        self._staged = None

    def stage(self, in_maps):
        from jax.sharding import NamedSharding, PartitionSpec
        concat = [
            np.ascontiguousarray(
                np.concatenate([np.asarray(m[n]) for m in in_maps], axis=0))
            for n in self.in_names
        ]
        sh = NamedSharding(self.mesh, PartitionSpec("core"))
        self._staged = [self.jax.device_put(a, sh) for a in concat]
        self.jax.block_until_ready(self._staged)

    def run(self):
        zeros = [
            np.zeros((N_CORES * z.shape[0], *z.shape[1:]), z.dtype)
            for z in self.zero_outs
        ]
        out = self.fn(*self._staged, *zeros)
        self.jax.block_until_ready(out)
        return [
            {
                n: np.asarray(out[i]).reshape(
                    N_CORES, *self.out_avals[i].shape)[c]
                for i, n in enumerate(self.out_names)
            }
            for c in range(N_CORES)
        ]


def _prep_inputs(embeddings, negative_embs, community_pos_options, neg_idx,
                 iter_n):
    embeddings = np.ascontiguousarray(np.asarray(embeddings, dtype=np.float32))
    negative_embs = np.ascontiguousarray(
        np.asarray(negative_embs, dtype=np.float32))
    cpo = np.asarray(community_pos_options)
    nidx = np.asarray(neg_idx)
    it = int(np.asarray(iter_n))
    pos_idx = cpo[:, it - 1].astype(np.int32)          # [N] rows of embeddings
    nidx = nidx.astype(np.int32)                       # [NUM_NEG, N]

    in_maps = []
    for c in range(N_CORES):
        base = c * N_SHARD
        esh = np.zeros((N_PAD, D), np.float32)
        esh[:N_SHARD] = embeddings[base:base + N_SHARD]
        pos_pad = np.zeros(N_PAD, np.int32)
        pos_pad[:N_SHARD] = pos_idx[base:base + N_SHARD]
        neg_pad = np.zeros((NUM_NEG, N_PAD), np.int32)
        neg_pad[:, :N_SHARD] = nidx[:, base:base + N_SHARD]
        idx = np.empty((128, NUM_NEG + 1, TILES), np.int32)
        # node j = t*128 + p  ->  idx[p, s, t]
        idx[:, 0, :] = pos_pad.reshape(TILES, 128).T
        idx[:, 1:, :] = neg_pad.reshape(NUM_NEG, TILES, 128).transpose(2, 0, 1)
        in_maps.append({
            "emb": embeddings,
            "negt": negative_embs,
            "esh": esh,
            "idx": idx,
        })
    return in_maps


def _get_runner():
    global _COMPILED
    if _COMPILED is None:
        _COMPILED = _Runner(_build_nc())
    return _COMPILED


def kernel(embeddings, negative_embs, community_pos_options, neg_idx, iter_n):
    r = _get_runner()
    in_maps = _prep_inputs(embeddings, negative_embs, community_pos_options,
                           neg_idx, iter_n)
    r.stage(in_maps)
    res = r.run()
    total = 0.0
    for c in range(N_CORES):
        sc = res[c]["scores"]              # [128, TILES], node j at [j%128, j//128]
        flat = sc.T.reshape(-1)            # node-ordered
        total += float(flat[:N_SHARD].astype(np.float64).sum())
    return np.float32(total / N)
